# revision 37
# baseline (speedup 1.0000x reference)
"""Trainium2 Bass kernel: pre-norm transformer block (dense_transformer).

Reference (per token row x of [4096, 768]):
  h1 = LN(x; g1, b1);  qkv = h1 @ w_qkv;  attention (12 heads, dh=64, softmax)
  x1 = x + attn_out @ w_proj + b_proj
  h2 = LN(x1; g2, b2); out = x1 + gelu(h2 @ w_fc1 + b_fc1) @ w_fc2 + b_fc2

Sharding: sequence (data) parallel - each of 8 cores owns 512 tokens.  K/V of
the full sequence are exchanged with three AllGathers (two head-pairs each,
fp8 payload); everything else is core-local.

Attention layout (feature-major activations, [feature, token]):
  scores:  S^T[k,q] = (Kpair^T[128, k-slice]).T @ Qblk[128, q]  - Qblk is the
           block-diagonal [Qa 0; 0 Qb] so ONE stationary tile serves both
           heads of the pair.
  softmax: exp on ACT (fp32 PSUM in, fp8 out), no max subtraction.
  AV:      fp8 DoubleRow matmul - V_aug [128, 2, 128] x P^T [128, 2, T]
           contracts 256 keys per instruction.  V_aug column 64 is ones, so
           acc row 64 accumulates sum-of-exp; columns 65:127 are pad
           (DoubleRow LDW requires all 128 weight columns).
  final:   O^T[0:64] * bcast(32/sumexp) (fp8 ao is scaled by 32 to stay in
           e4m3 normal range; w_proj absorbs the 1/32).

QKV and proj matmuls run as fp8 DoubleRow too (weights scaled by 32 on the
host against e4m3 subnormals; the de-scale rides the existing bias-add).
The MLP stays bf16: fp8 there would push the residual-stream error past the
correctness gate.  LN statistics / softmax sums / residuals stay fp32.
"""

import os
import sys

import numpy as np

for _p in ("/opt/trn_rl_repo",):
    if os.path.isdir(_p) and _p not in sys.path:
        sys.path.insert(0, _p)

os.environ.setdefault("MYCRO_LOCAL_CACHE", "1")

import ml_dtypes  # noqa: E402

import concourse.bass as bass  # noqa: E402
import concourse.mybir as mybir  # noqa: E402
import concourse.tile as tile  # noqa: E402
from concourse import bacc  # noqa: E402

DIM = 768
N_TOK = 4096
HEADS = 12
DH = 64
HIDDEN = 4 * DIM
EPS = 1e-5
N_CORES = 8
T = N_TOK // N_CORES          # 512 local tokens per core
P = 128
CT = DIM // P                 # 6 feature tiles
CT2 = CT // 2                 # 3 DoubleRow contraction chunks
KT = N_TOK // P               # 32 key tiles
KTP = KT // 2                 # 16 key tile pairs (DoubleRow granularity)
LPC = T // P                  # 4 local token tiles
SCALE = DH ** -0.5
PAIRS = HEADS // 2
WS = 32.0                     # fp8 weight scale (e4m3 subnormal guard)
AOS = 32.0                    # fp8 attention-out scale

F32 = mybir.dt.float32
BF16 = mybir.dt.bfloat16
FP8 = mybir.dt.float8e4
AF = mybir.ActivationFunctionType
ALU = mybir.AluOpType
DR = mybir.MatmulPerfMode.DoubleRow

KSZ = P * T                   # 65536: K^T shard elems (fp8)
VSZ = P * 2 * 2 * 2 * (DH + 1)  # 66560: V shard elems incl. ones col (fp8)
PRSZ = KSZ + VSZ
GSIZES = (1, 2, 3)            # pairs per collective: first fires earliest
NGATH = len(GSIZES)
GOFF = []                     # pair -> (gather idx, offset within gather)
for _g, _n in enumerate(GSIZES):
    for _o in range(_n):
        GOFF.append((_g, _o))

_CACHED_NC = None
LAST_RESULTS = None


def build_nc():
    nc = bacc.Bacc(num_devices=N_CORES)

    xt = nc.declare_dram_parameter("xt", [DIM, T], F32, isOutput=False)
    wqkv = nc.declare_dram_parameter("wqkv", [DIM, 3 * DIM], FP8, isOutput=False)
    biases = nc.declare_dram_parameter("biases", [P, 48], F32, isOutput=False)
    bvbc = nc.declare_dram_parameter("bvbc", [P, DIM], F32, isOutput=False)
    wproj = nc.declare_dram_parameter("wproj", [DIM, DIM], FP8, isOutput=False)
    wfc1 = nc.declare_dram_parameter("wfc1", [DIM, HIDDEN], FP8, isOutput=False)
    wfc2 = nc.declare_dram_parameter("wfc2", [HIDDEN, DIM], BF16, isOutput=False)
    outt = nc.declare_dram_parameter("outt", [DIM, T], F32, isOutput=True)

    with tile.TileContext(nc) as tc:
        _emit(nc, tc, xt, wqkv, biases, bvbc, wproj, wfc1, wfc2, outt)
    nc.finalize()
    return nc


def _emit(nc, tc, xt, wqkv, biases, bvbc, wproj, wfc1, wfc2, outt):
    from contextlib import ExitStack

    top = ExitStack()

    def pool(name, bufs, space="SBUF", stack=None):
        return (stack or top).enter_context(
            tc.tile_pool(name=name, bufs=bufs, space=space))

    # ---- long-lived SBUF pools ----
    const = pool("const", 1)
    xpool = pool("x", 1)               # x^T fp32, lives to the proj residual
    hpool = pool("h", 1)               # LN temporaries + h2
    h1pool = pool("h1", 1)             # h1 fp8 DoubleRow pairs
    qbig = pool("qbig", 1)             # block-diagonal Q fp8 per pair
    kf8 = pool("kf8", 2)               # K band fp8 staging before send
    vf8 = pool("vf8", 4)               # V tile fp8 staging before send
    kpair = pool("kpair", 2)           # gathered K^T fp8 [128, 4096]
    vpair = pool("vpair", 2)           # gathered V_aug fp8
    aopool = pool("ao", 1)             # attention out fp8 DoubleRow pairs
    x1pool = pool("x1", 1)             # post-attention residual fp32
    gpool = pool("g", 12)              # gelu activations bf16
    opool = pool("o", 3)               # output fp32 staging
    stat = pool("stat", 1)             # small statistics tiles
    ptpool = pool("pt", 5)             # P^T = exp(scores) fp8 [128,2,2,T]
    dram = pool("dram", 1, space="DRAM")

    # warm-up: a tiny AllGather so peer start-skew is absorbed here, not at
    # the first real K/V gather
    warm_in = dram.tile([64], FP8, name="warmi")
    warm_out = dram.tile([N_CORES, 64], FP8, name="warmo", addr_space="Shared")
    nc.gpsimd.collective_compute(
        "AllGather", ALU.bypass, replica_groups=[list(range(N_CORES))],
        ins=[warm_in[:]], outs=[warm_out[:, :]])

    # ---- x^T and qkv weights first: they gate the K/V gathers ----
    x_sb = [xpool.tile([P, T], F32, name=f"x{t}") for t in range(CT)]
    for t in range(CT):
        nc.sync.dma_start(x_sb[t][:], xt[t * P:(t + 1) * P, :])

    pA = ExitStack()
    qkvw = pool("qkvw", 3, stack=pA)   # qkv DR bands (freed after Q)
    qkv_dr = []
    for b in range(CT2):
        w = qkvw.tile([P, 2, 3 * DIM], FP8, tag="wband", name=f"wb{b}")
        nc.sync.dma_start(
            w[:], wqkv[2 * b * P:(2 * b + 2) * P, :].rearrange(
                "(two p) w -> p two w", two=2))
        qkv_dr.append(w)

    # ---- constants / bias vectors ----
    ones_stat = const.tile([P, 1], BF16)
    nc.vector.memset(ones_stat[:], 1.0)
    ones_stat32 = const.tile([P, 1], F32)
    nc.vector.memset(ones_stat32[:], 1.0)
    fill_row = const.tile([1, T], F32)
    nc.vector.memset(fill_row[:], 0.0)
    ones_row = const.tile([1, P], BF16)
    nc.vector.memset(ones_row[:], 1.0)
    row_aos = const.tile([1, DH], BF16)
    nc.vector.memset(row_aos[:], AOS)
    zero_bias = const.tile([P, 1], F32)
    nc.vector.memset(zero_bias[:], 0.0)
    ones_f8 = const.tile([P, 4], FP8)
    nc.vector.memset(ones_f8[:], 1.0)
    eps_tile = const.tile([1, 1], F32)
    nc.vector.memset(eps_tile[:], EPS)

    bias_sb = const.tile([P, 48], F32)
    nc.sync.dma_start(bias_sb[:], biases[:, :])
    bqk_sb = bias_sb[:, 0:12]
    bproj_sb = bias_sb[:, 12:18]
    bfc1_sb = bias_sb[:, 18:42]
    bfc2_sb = bias_sb[:, 42:48]
    bv_bc = const.tile([P, DIM], F32)
    nc.sync.dma_start(bv_bc[:], bvbc[:, :])

    # ---- layernorm helpers ----
    def ln_finish(s_ps, sq_ps, bcps):
        """Turn accumulated sum / sum-of-squares into broadcast rstd and
        mean*rstd PSUM tiles."""
        ssum = stat.tile([1, T], F32, name="lnsum")
        nc.vector.tensor_copy(ssum[:], s_ps[:])
        t1 = stat.tile([1, T], F32, name="lnt1")
        nc.vector.scalar_tensor_tensor(t1[:], ssum[:], 1.0 / DIM, ssum[:],
                                       ALU.mult, ALU.mult)
        t2 = stat.tile([1, T], F32, name="lnt2")
        nc.vector.tensor_sub(t2[:], sq_ps[:], t1[:])
        sdev = stat.tile([1, T], F32, name="lnsdev")
        nc.scalar.activation(sdev[:], t2[:], AF.Sqrt,
                             bias=eps_tile[:], scale=1.0 / DIM)
        rstd = stat.tile([1, T], F32, name="lnrstd")
        nc.vector.reciprocal_approx_fast(rstd[:], sdev[:])
        rstd_b = stat.tile([1, T], BF16, name="lnrstdb")
        nc.vector.tensor_copy(rstd_b[:], rstd[:])
        mrs_b = stat.tile([1, T], BF16, name="lnmrsb")
        nc.vector.scalar_tensor_tensor(mrs_b[:], ssum[:], 1.0 / DIM, rstd[:],
                                       ALU.mult, ALU.mult)
        rstd_ps = bcps.tile([P, T], F32, tag="bc")
        nc.tensor.matmul(rstd_ps[:], ones_row[:], rstd_b[:], start=True, stop=True)
        mrs_ps = bcps.tile([P, T], F32, tag="bc")
        nc.tensor.matmul(mrs_ps[:], ones_row[:], mrs_b[:], start=True, stop=True)
        rstd_sb = stat.tile([P, T], F32, tag="rstdsb", bufs=2, name="rstdsb")
        nc.vector.tensor_copy(rstd_sb[:], rstd_ps[:])
        mrs_sb = stat.tile([P, T], F32, tag="mrssb", bufs=2, name="mrssb")
        nc.vector.tensor_copy(mrs_sb[:], mrs_ps[:])
        return rstd_sb, mrs_sb

    def layernorm(src_tiles, nm, stps, bcps, dsts):
        """dsts[t] is the destination AP for normalized tile t (any dtype)."""
        s_ps = stps.tile([1, T], F32, tag="s")
        sq_ps = stps.tile([1, T], F32, tag="sq")
        for t in range(CT):
            xb = hpool.tile([P, T], BF16, tag="lnxb", bufs=2, name=f"{nm}xb{t}")
            nc.vector.tensor_copy(xb[:], src_tiles[t][:])
            xsq = hpool.tile([P, T], BF16, tag="lnxsq", bufs=2, name=f"{nm}sq{t}")
            nc.vector.tensor_mul(xsq[:], xb[:], xb[:])
            nc.tensor.matmul(s_ps[:], ones_stat[:], xb[:],
                             start=(t == 0), stop=(t == CT - 1))
            nc.tensor.matmul(sq_ps[:], ones_stat[:], xsq[:],
                             start=(t == 0), stop=(t == CT - 1))
        rstd_sb, mrs_sb = ln_finish(s_ps, sq_ps, bcps)
        for t in range(CT):
            tmp = hpool.tile([P, T], F32, tag="lntmp", bufs=2, name=f"{nm}tm{t}")
            nc.vector.tensor_mul(tmp[:], src_tiles[t][:], rstd_sb[:])
            nc.vector.tensor_sub(dsts[t], tmp[:], mrs_sb[:])

    # ======================= phase A: LN1, QKV, gathers ======================
    stpsA = pool("stpsA", 1, space="PSUM", stack=pA)
    bcpsA = pool("bcpsA", 2, space="PSUM", stack=pA)
    mmpsA = pool("mmpsA", 2, space="PSUM", stack=pA)
    vps = pool("vps", 2, space="PSUM", stack=pA)

    # h1 in fp8 DoubleRow pair layout [p][band parity][t]
    h1big = [h1pool.tile([P, 2, T], FP8, name=f"h1b{b}") for b in range(CT2)]
    layernorm(x_sb, "h1", stpsA, bcpsA,
              [h1big[t // 2][:, t % 2, :] for t in range(CT)])

    def qk_band(m):
        """QKV projection band m (fp8 DR), returns PSUM [P, T]."""
        ps = mmpsA.tile([P, T], F32, tag="mm")
        for b in range(CT2):
            nc.tensor.matmul(ps[:], qkv_dr[b][:, :, m * P:(m + 1) * P],
                             h1big[b][:, :, :],
                             start=(b == 0), stop=(b == CT2 - 1), perf_mode=DR)
        return ps

    # K band + V slice per pair; one AllGather per two pairs
    kv_in = []
    kv_out = []
    for g in range(NGATH):
        kv_in.append(dram.tile([GSIZES[g] * PRSZ], FP8, name=f"kvi{g}"))
        kv_out.append(dram.tile([N_CORES, GSIZES[g] * PRSZ], FP8,
                                name=f"kvo{g}", addr_space="Shared"))

    for pr in range(PAIRS):
        g, off = GOFF[pr]
        kv_in_pr = kv_in[g][off * PRSZ:(off + 1) * PRSZ]

        m = CT + pr
        ps = qk_band(m)
        k_f8 = kf8.tile([P, T], FP8, tag="k", name=f"k{pr}")
        nc.vector.tensor_scalar(k_f8[:], ps[:], 1.0 / WS, bqk_sb[:, m:m + 1],
                                ALU.mult, ALU.add)
        nc.sync.dma_start(kv_in_pr[0:KSZ].rearrange("(p t) -> p t", t=T), k_f8[:])

        # V slice for this pair, token-major; payload [p][kp_l][h][par][d+ones]
        vview = kv_in_pr[KSZ:PRSZ].rearrange(
            "(p kp h par d) -> p kp h par d", kp=2, h=2, par=2, d=DH + 1)
        for mt in range(LPC):
            vp = vps.tile([P, 2 * DH], F32, tag="vps")
            for b in range(CT2):
                nc.tensor.matmul(
                    vp[:],
                    h1big[b][:, :, mt * P:(mt + 1) * P],
                    qkv_dr[b][:, :, 2 * DIM + 2 * pr * DH:2 * DIM + (2 * pr + 2) * DH],
                    start=(b == 0), stop=(b == CT2 - 1), perf_mode=DR)
            v_f8 = vf8.tile([P, 2, DH + 1], FP8, tag="v", name=f"v{pr}_{mt}")
            nc.vector.memset(v_f8[:, :, DH:DH + 1], 1.0)
            nc.vector.scalar_tensor_tensor(
                v_f8[:, :, 0:DH], vp[:].rearrange("p (h d) -> p h d", d=DH),
                1.0 / WS,
                bv_bc[:, 2 * pr * DH:(2 * pr + 2) * DH].rearrange(
                    "p (h d) -> p h d", d=DH),
                ALU.mult, ALU.add)
            nc.sync.dma_start(vview[:, mt // 2, :, mt % 2, :], v_f8[:])

        if off == GSIZES[g] - 1:
            nc.gpsimd.collective_compute(
                "AllGather", ALU.bypass,
                replica_groups=[list(range(N_CORES))],
                ins=[kv_in[g][:]], outs=[kv_out[g][:, :]])

    # block-diagonal Q tiles: zero the off-diagonal quadrants (emitted here so
    # the DVE does LN1 + K/V casts first - these memsets aren't urgent)
    q_big = [qbig.tile([P, 2 * T], FP8, name=f"qb{pr}") for pr in range(PAIRS)]
    for pr in range(PAIRS):
        nc.vector.memset(q_big[pr][DH:P, 0:T], 0.0)
        nc.vector.memset(q_big[pr][0:DH, T:2 * T], 0.0)

    # Q projections run while gathers are in flight
    for pr in range(PAIRS):
        ps = qk_band(pr)
        nc.vector.tensor_scalar(q_big[pr][0:DH, 0:T], ps[0:DH, :], 1.0 / WS,
                                bqk_sb[0:DH, pr:pr + 1], ALU.mult, ALU.add)
        nc.vector.tensor_scalar(q_big[pr][DH:P, T:2 * T], ps[DH:P, :], 1.0 / WS,
                                bqk_sb[DH:P, pr:pr + 1], ALU.mult, ALU.add)

    fill_ps = mmpsA.tile([1, T], F32, tag="mm", name="fill")
    for _ in range(40):
        nc.tensor.matmul(fill_ps[:], ones_stat32[0:1, 0:1], fill_row[:],
                         start=True, stop=True)

    pA.close()

    def load_kpair(pr):
        g, off = GOFF[pr]
        kt_ = kpair.tile([P, N_TOK], FP8, tag="kp", name=f"kp{pr}")
        kv = kv_out[g][:, off * PRSZ:off * PRSZ + KSZ].rearrange(
            "c (p t) -> p c t", p=P)
        kv_sb = kt_[:].rearrange("p (c t) -> p c t", c=N_CORES)
        nc.sync.dma_start(kv_sb[:, 0:2, :], kv[:, 0:2, :])
        nc.sync.dma_start(kv_sb[:, 2:N_CORES, :], kv[:, 2:N_CORES, :])
        return kt_

    def load_vpair(pr):
        # [p][h][c][kp_l][par][128]: d 0:64 = V, 64 = ones, 65:128 = unused pad
        # (DoubleRow LDW requires full 128 weight columns + 16B subtile stride)
        g, off = GOFF[pr]
        vt = vpair.tile([P, 2, N_CORES, 2, 2, P], FP8, tag="vp", name=f"vp{pr}")
        src = kv_out[g][:, off * PRSZ + KSZ:(off + 1) * PRSZ].rearrange(
            "c (p kp h par d) -> p c kp h par d", p=P, kp=2, h=2, par=2, d=DH + 1)
        for h in range(2):
            for kp in range(2):
                for par in range(2):
                    nc.sync.dma_start(vt[:, h, :, kp, par, 0:DH + 1],
                                      src[:, :, kp, h, par, :])
        return vt

    # ======================= phase B: attention ==============================
    pB = ExitStack()
    scps = pool("scps", 2, space="PSUM", stack=pB)
    accps = pool("accps", 4, space="PSUM", stack=pB)

    DELAY = 4  # kt iterations of the next pair emitted before prev pair's
    #            normalization, to hide the reciprocal chain latency

    kt_tiles = {0: load_kpair(0), 1: load_kpair(1)}
    vt_tiles = {0: load_vpair(0), 1: load_vpair(1)}

    # prefetch phase-C weights; DMAs run during attention
    projw = pool("projw", 3)
    fc1w = pool("fc1w", 3)
    fc2w = pool("fc2w", 12)
    proj_dr = []
    for b in range(CT2):
        w = projw.tile([P, 2, DIM], FP8, tag="wband", name=f"pj{b}")
        nc.gpsimd.dma_start(
            w[:], wproj[2 * b * P:(2 * b + 2) * P, :].rearrange(
                "(two p) w -> p two w", two=2))
        proj_dr.append(w)

    def wband(pool_, kt, src, width, nm):
        b = pool_.tile([P, width], BF16, tag="wband", name=nm)
        nc.gpsimd.dma_start(b[:], src[kt * P:(kt + 1) * P, :])
        return b

    fc1_dr = []
    for b in range(CT2):
        w = fc1w.tile([P, 2, HIDDEN], FP8, tag="wband", name=f"f1{b}")
        nc.gpsimd.dma_start(
            w[:], wfc1[2 * b * P:(2 * b + 2) * P, :].rearrange(
                "(two p) w -> p two w", two=2))
        fc1_dr.append(w)
    fc2_bands = [wband(fc2w, kt, wfc2, DIM, f"f2{kt}") for kt in range(12)]

    ao_big = [aopool.tile([P, 2, T], FP8, name=f"ao{b}") for b in range(CT2)]
    x1_sb = [x1pool.tile([P, T], F32, name=f"x1_{m}") for m in range(CT)]

    def proj_chunk(b):
        """One DoubleRow contraction chunk of the output projection, run as
        soon as ao_big[b] (pairs 2b, 2b+1) is normalized.  Accumulates the
        raw (scaled) partial products into x1_sb via the DVE."""
        for m in range(CT):
            ps = accps.tile([P, T], F32, tag="acc", name=f"pj{b}_{m}")
            nc.tensor.matmul(ps[:], proj_dr[b][:, :, m * P:(m + 1) * P],
                             ao_big[b][:, :, :], start=True, stop=True,
                             perf_mode=DR)
            if b == 0:
                nc.vector.tensor_scalar(x1_sb[m][:], ps[:], 1.0 / (WS * AOS),
                                        bproj_sb[:, m:m + 1], ALU.mult, ALU.add)
            else:
                nc.vector.scalar_tensor_tensor(x1_sb[m][:], ps[:],
                                               1.0 / (WS * AOS), x1_sb[m][:],
                                               ALU.mult, ALU.add)

    pending = None  # previous pair's deferred normalization
    for pr in range(PAIRS):
        if pr not in kt_tiles:
            kt_tiles[pr] = load_kpair(pr)
            vt_tiles[pr] = load_vpair(pr)
        k_tile = kt_tiles.pop(pr)
        v_tile = vt_tiles.pop(pr)
        q_tile = q_big[pr]
        acc_a = accps.tile([P, T], F32, tag="acc", name=f"acca{pr}")
        acc_b = accps.tile([P, T], F32, tag="acc", name=f"accb{pr}")

        pts = {}

        def sc_exp(kt, pr=pr, k_tile=k_tile, q_tile=q_tile, pts=pts):
            ktp, par = divmod(kt, 2)
            sc = scps.tile([P, 2 * T], F32, tag="sc", name=f"sc{pr}_{kt}")
            st = k_tile[:, kt * P:(kt + 1) * P]
            nc.tensor.matmul(sc[:, 0:T], st, q_tile[:, 0:T], start=True, stop=True)
            nc.tensor.matmul(sc[:, T:2 * T], st, q_tile[:, T:2 * T],
                             start=True, stop=True)
            if par == 0:
                pts[ktp] = ptpool.tile([P, 2, 2, T], FP8, tag="pt",
                                       name=f"pt{pr}_{ktp}")
            nc.scalar.activation(pts[ktp][:, :, par, :],
                                 sc[:].rearrange("p (h t) -> p h t", h=2),
                                 AF.Exp, bias=zero_bias[:], scale=SCALE)

        def av(ktp, v_tile=v_tile, acc_a=acc_a, acc_b=acc_b, pts=pts):
            pt = pts.pop(ktp)
            for h, acc in ((0, acc_a), (1, acc_b)):
                nc.tensor.matmul(
                    acc[:], v_tile[:, h, ktp // 2, ktp % 2, :, :],
                    pt[:, h, :, :],
                    start=(ktp == 0), stop=(ktp == KTP - 1), perf_mode=DR)

        # prologue: scores+exp only, so the previous pair's normalization
        # (emitted below) binds its PSUM tiles before this pair's first AV.
        # AVs trail the exp stream by LAGK kt-pairs so an AV stall (V-tile
        # DMA, fin/proj PSUM rotation) never blocks scores in the in-order
        # PE queue.
        LAGK = 4
        for kt in range(DELAY):
            sc_exp(kt)
        if pending is not None:
            pending()
            if pr >= 2 and pr % 2 == 0:
                proj_chunk(pr // 2 - 1)
        for kt in range(DELAY, KT):
            sc_exp(kt)
            if kt % 2 == 1 and (kt - 1) // 2 >= LAGK:
                av((kt - 1) // 2 - LAGK)
        for j in range(KTP - LAGK, KTP):
            av(j)

        def mk_finish(pr, acc_a, acc_b):
            def fin():
                for half, acc in ((0, acc_a), (1, acc_b)):
                    se = stat.tile([1, T], F32, tag="se", bufs=2,
                                   name=f"se{pr}{half}")
                    nc.vector.tensor_copy(se[:], acc[DH:DH + 1, :])
                    rec = stat.tile([1, T], F32, tag="rec", bufs=2,
                                    name=f"rc{pr}{half}")
                    nc.vector.reciprocal_approx_fast(rec[:], se[:])
                    rec_b = stat.tile([1, T], BF16, tag="recb", bufs=2,
                                      name=f"rb{pr}{half}")
                    nc.vector.tensor_copy(rec_b[:], rec[:])
                    bc = accps.tile([DH, T], F32, tag="acc", name=f"bc{pr}{half}")
                    nc.tensor.matmul(bc[:], row_aos[:], rec_b[:],
                                     start=True, stop=True)
                    bc_sb = stat.tile([DH, T], F32, tag="bcsb", bufs=2,
                                      name=f"bs{pr}{half}")
                    nc.vector.tensor_copy(bc_sb[:], bc[:])
                    # head (2*pr + half) = rows half*DH:(half+1)*DH of feature
                    # band pr = slice [:, pr % 2, :] of ao_big[pr // 2]
                    nc.vector.tensor_mul(
                        ao_big[pr // 2][half * DH:(half + 1) * DH, pr % 2, :],
                        acc[0:DH, :], bc_sb[:])
            return fin

        pending = mk_finish(pr, acc_a, acc_b)

    pending()
    proj_chunk(CT2 - 1)
    pB.close()

    # ============ phase C1: x1 finalize + LN2 ================================
    pC1 = ExitStack()
    stpsC = pool("stpsC", 1, space="PSUM", stack=pC1)
    bcpsC = pool("bcpsC", 2, space="PSUM", stack=pC1)

    s_ps2 = stpsC.tile([1, T], F32, tag="s")
    sq_ps2 = stpsC.tile([1, T], F32, tag="sq")
    for m in range(CT):
        nc.vector.tensor_add(x1_sb[m][:], x1_sb[m][:], x_sb[m][:])
        xsq = hpool.tile([P, T], BF16, tag="lnxsq", bufs=2, name=f"h2sq{m}")
        nc.scalar.square(xsq[:], x1_sb[m][:])
        nc.tensor.matmul(s_ps2[:], ones_stat32[:], x1_sb[m][:],
                         start=(m == 0), stop=(m == CT - 1))
        nc.tensor.matmul(sq_ps2[:], ones_stat[:], xsq[:],
                         start=(m == 0), stop=(m == CT - 1))

    rstd_sb, mrs_sb = ln_finish(s_ps2, sq_ps2, bcpsC)
    h2big = [h1pool.tile([P, 2, T], FP8, name=f"h2b{b}") for b in range(CT2)]
    for t in range(CT):
        tmp = hpool.tile([P, T], F32, tag="lntmp", bufs=2, name=f"h2tm{t}")
        nc.vector.tensor_mul(tmp[:], x1_sb[t][:], rstd_sb[:])
        nc.vector.tensor_sub(h2big[t // 2][:, t % 2, :], tmp[:], mrs_sb[:])
    pC1.close()

    # ======================= phase C2: MLP ===================================
    pC2 = ExitStack()
    mmpsM = pool("mmpsM", 2, space="PSUM", stack=pC2)
    fc2ps = pool("fc2ps", CT, space="PSUM", stack=pC2)

    g_sb = [gpool.tile([P, T], BF16, tag="g", name=f"g{m}")
            for m in range(HIDDEN // P)]
    for m in range(HIDDEN // P):
        ps = mmpsM.tile([P, T], F32, tag="mm")
        for b in range(CT2):
            nc.tensor.matmul(ps[:], fc1_dr[b][:, :, m * P:(m + 1) * P],
                             h2big[b][:, :, :],
                             start=(b == 0), stop=(b == CT2 - 1), perf_mode=DR)
        nc.scalar.activation(g_sb[m][:], ps[:], AF.Gelu,
                             bias=bfc1_sb[:, m:m + 1], scale=1.0 / WS)

    o_ps = [fc2ps.tile([P, T], F32, tag="oacc", name=f"ops{m}") for m in range(CT)]
    NKT2 = HIDDEN // P
    bands = {}
    for kt in range(NKT2 - CT):
        band = fc2_bands[kt] if kt < 12 else wband(fc2w, kt, wfc2, DIM, f"f2{kt}")
        bands[kt] = band
        for m in range(CT):
            nc.tensor.matmul(o_ps[m][:], band[:, m * P:(m + 1) * P], g_sb[kt][:],
                             start=(kt == 0), stop=False)
    for kt in range(NKT2 - CT, NKT2):
        bands[kt] = wband(fc2w, kt, wfc2, DIM, f"f2{kt}")
    for m in range(CT):
        for kt in range(NKT2 - CT, NKT2):
            nc.tensor.matmul(o_ps[m][:], bands[kt][:, m * P:(m + 1) * P],
                             g_sb[kt][:], start=False, stop=(kt == NKT2 - 1))
        ot = opool.tile([P, T], F32, tag="ot", name=f"ot{m}")
        nc.vector.scalar_tensor_tensor(ot[:], o_ps[m][:], bfc2_sb[:, m:m + 1],
                                       x1_sb[m][:], ALU.add, ALU.add)
        nc.sync.dma_start(outt[m * P:(m + 1) * P, :], ot[:])
    pC2.close()
    top.close()


def _prepare_in_maps(inputs):
    x = np.asarray(inputs["x"], np.float32)
    g1 = np.asarray(inputs["g1"], np.float32)
    b1 = np.asarray(inputs["b1"], np.float32)
    g2 = np.asarray(inputs["g2"], np.float32)
    b2 = np.asarray(inputs["b2"], np.float32)
    w_qkv = np.asarray(inputs["w_qkv"], np.float32)
    w_proj = np.asarray(inputs["w_proj"], np.float32)
    b_proj = np.asarray(inputs["b_proj"], np.float32)
    w_fc1 = np.asarray(inputs["w_fc1"], np.float32)
    b_fc1 = np.asarray(inputs["b_fc1"], np.float32)
    w_fc2 = np.asarray(inputs["w_fc2"], np.float32)
    b_fc2 = np.asarray(inputs["b_fc2"], np.float32)

    bf = ml_dtypes.bfloat16
    f8 = ml_dtypes.float8_e4m3
    wqkv_eff = (g1[:, None] * w_qkv * WS).astype(f8)
    bqkv_eff = (b1 @ w_qkv).astype(np.float32)
    wfc1_eff = (g2[:, None] * w_fc1 * WS).astype(f8)
    bfc1_eff = (b_fc1 + b2 @ w_fc1).astype(np.float32)

    bias_pack = np.concatenate([
        bqkv_eff[:2 * DIM].reshape(2 * DIM // P, P).T,
        b_proj.reshape(CT, P).T,
        bfc1_eff.reshape(HIDDEN // P, P).T,
        b_fc2.reshape(CT, P).T,
    ], axis=1).astype(np.float32)
    shared = {
        "wqkv": wqkv_eff,
        "biases": np.ascontiguousarray(bias_pack),
        "bvbc": np.ascontiguousarray(
            np.broadcast_to(bqkv_eff[2 * DIM:], (P, DIM))).astype(np.float32),
        "wproj": (w_proj * WS).astype(f8),
        "wfc1": wfc1_eff,
        "wfc2": w_fc2.astype(bf),
    }
    in_maps = []
    for c in range(N_CORES):
        xs = np.ascontiguousarray(x[0, c * T:(c + 1) * T, :].T)
        in_maps.append({"xt": xs, **shared})
    return in_maps


def _install_ntff_hook():
    """The agent image's antenv lacks axon_hooks; synthesize it so
    BASS_TRACE=1 profiling works (and its absence never crashes)."""
    import types
    try:
        from antenv.axon_hooks import get_axon_ntff_profile_hook  # noqa: F401
        return
    except ImportError:
        pass
    try:
        import antenv
        mod = types.ModuleType("antenv.axon_hooks")
        _h = [None]
        mod.set_axon_ntff_profile_hook = lambda h: _h.__setitem__(0, h)
        mod.get_axon_ntff_profile_hook = lambda: _h[0]
        sys.modules["antenv.axon_hooks"] = mod
        antenv.axon_hooks = mod
        try:
            from trn_agent_boot.trn_boot import _ntff_profile_via_ctypes
            so = "/opt/axon/libaxon_pjrt.so"
            if os.path.exists(so):
                mod.set_axon_ntff_profile_hook(_ntff_profile_via_ctypes(so))
        except Exception:
            pass
    except Exception:
        pass


def kernel(**inputs):
    global _CACHED_NC, LAST_RESULTS
    from concourse.bass_utils import run_bass_kernel_spmd

    _install_ntff_hook()

    if _CACHED_NC is None:
        _CACHED_NC = build_nc()
    nc = _CACHED_NC
    in_maps = _prepare_in_maps(inputs)
    res = run_bass_kernel_spmd(nc, in_maps, list(range(N_CORES)))
    LAST_RESULTS = res
    out = np.empty((1, N_TOK, DIM), np.float32)
    for c in range(N_CORES):
        out[0, c * T:(c + 1) * T, :] = res.results[c]["outt"].T
    return out
